# revision 9
# baseline (speedup 1.0000x reference)
# Trainium2 Bass kernel for nn_ComplexMeasurement: out[b,n] = Re(z_n^T Z_b z_n)
# with z = kr + i*ki (rows of the measurement kernel), Z = input_real + i*input_imag.
#
# Per batch b:  P = kr @ R_b - ki @ I_b   [U, D]   (PSUM-accumulated matmuls)
#               Q = ki @ R_b + kr @ I_b   [U, D]
#               out[b, n] = sum_j P[n,j]*kr[n,j] - Q[n,j]*ki[n,j]
#
# Sharding: data-parallel over batch, 16 batches per core on 8 cores.
# Stationary operands (kr^T, ki^T, -ki^T tiles) are pre-transposed on host.
import sys

for _p in ("/opt/trn_rl_repo", "/opt/trn_rl_repo/concourse"):
    if _p not in sys.path:
        sys.path.insert(0, _p)

import numpy as np

B, U, D = 128, 200, 512
NCORES = 8
BPC = B // NCORES  # batches per core
NT = D // 128  # contraction tiles
CHUNKS = ((0, 128), (128, 72))  # n-dim chunks (start, width)


def _split_multi_waits(nc, max_waits=1):
    # walrus in this env rejects instructions carrying >1 semaphore wait
    # ("Too many sync wait commands"). Move excess waits onto single-wait
    # NoOps inserted directly before the instruction on the same engine.
    import concourse.mybir as mybir

    n = 0
    for f in nc.m.functions:
        for bb in f.blocks:
            out = []
            changed = False
            for inst in bb.instructions:
                si = getattr(inst, "sync_info", None)
                waits = list(si.on_wait) if si is not None and si.on_wait else []
                if len(waits) > max_waits:
                    changed = True
                    extra, keep = waits[:-max_waits], waits[-max_waits:]
                    for w in extra:
                        n += 1
                        out.append(
                            mybir.InstNoOp(
                                name=f"WSPLIT-{n}",
                                engine=inst.engine,
                                ins=[],
                                outs=[],
                                sync_info=mybir.SyncInfo(on_wait=[w], on_update=[]),
                            )
                        )
                    inst.sync_info = mybir.SyncInfo(
                        on_wait=keep, on_update=list(si.on_update)
                    )
                out.append(inst)
            if changed:
                bb.instructions = out
    return n


def build_nc():
    import concourse.bass as bass
    import concourse.mybir as mybir
    import concourse.tile as tile

    f32 = mybir.dt.float32
    f32r = mybir.dt.float32r
    mult = mybir.AluOpType.mult
    add = mybir.AluOpType.add

    nc = bass.Bass()
    zr_d = nc.declare_dram_parameter("zr", [BPC, D, D], f32, isOutput=False)
    zi_d = nc.declare_dram_parameter("zi", [BPC, D, D], f32, isOutput=False)
    wkr_d = nc.declare_dram_parameter("wkr", [NT, 128, U], f32, isOutput=False)
    wki_d = nc.declare_dram_parameter("wki", [NT, 128, U], f32, isOutput=False)
    wkin_d = nc.declare_dram_parameter("wkin", [NT, 128, U], f32, isOutput=False)
    kq_d = nc.declare_dram_parameter("kq", [U, 2 * D], f32, isOutput=False)
    out_d = nc.declare_dram_parameter("out", [U, BPC], f32, isOutput=True)

    with tile.TileContext(nc) as tc:
        with (
            tc.tile_pool(name="const", bufs=1) as constp,
            tc.tile_pool(name="io", bufs=4) as iop,
            tc.tile_pool(name="scr", bufs=2) as scrp,
            tc.tile_pool(name="ps", bufs=2, space="PSUM") as psp,
        ):
            wkr = constp.tile([128, NT, U], f32r)
            wki = constp.tile([128, NT, U], f32r)
            wkin = constp.tile([128, NT, U], f32r)
            nc.sync.dma_start(wkr[:], wkr_d[:].bitcast(f32r).rearrange("t p n -> p t n"))
            nc.gpsimd.dma_start(wki[:], wki_d[:].bitcast(f32r).rearrange("t p n -> p t n"))
            nc.scalar.dma_start(wkin[:], wkin_d[:].bitcast(f32r).rearrange("t p n -> p t n"))
            kqc = {}
            outc = {}
            for ci, (cs, cw) in enumerate(CHUNKS):
                kqc[ci] = constp.tile([cw, 2 * D], f32, tag=f"kqc{ci}", name=f"kqc{ci}")
                nc.scalar.dma_start(kqc[ci][:], kq_d[cs : cs + cw, :])
                outc[ci] = constp.tile([cw, BPC], f32, tag=f"out{ci}", name=f"outc{ci}")

            for b in range(BPC):
                r_sb = iop.tile([128, NT, D], f32r, tag="r")
                i_sb = iop.tile([128, NT, D], f32r, tag="i")
                r_eng = (nc.sync, nc.scalar, nc.sync, nc.gpsimd)
                i_eng = (nc.gpsimd, nc.sync, nc.gpsimd, nc.scalar)
                for t in range(NT):
                    r_eng[t].dma_start(
                        r_sb[:, t, :],
                        zr_d[b, 128 * t : 128 * (t + 1), :].bitcast(f32r),
                    )
                    i_eng[t].dma_start(
                        i_sb[:, t, :],
                        zi_d[b, 128 * t : 128 * (t + 1), :].bitcast(f32r),
                    )
                for ci, (cs, cw) in enumerate(CHUNKS):
                    sl = slice(cs, cs + cw)
                    pPQ = psp.tile([cw, 2 * D], mybir.dt.float32, tag=f"PQ{ci}")
                    pP = pPQ[:, 0:D]
                    pQ = pPQ[:, D : 2 * D]
                    for t in range(NT):
                        nc.tensor.matmul(
                            pP, wkr[:, t, sl], r_sb[:, t, :],
                            start=(t == 0), stop=False,
                        )
                    for t in range(NT):
                        nc.tensor.matmul(
                            pP, wkin[:, t, sl], i_sb[:, t, :],
                            start=False, stop=(t == NT - 1),
                        )
                    for t in range(NT):
                        nc.tensor.matmul(
                            pQ, wki[:, t, sl], r_sb[:, t, :],
                            start=(t == 0), stop=False,
                        )
                    for t in range(NT):
                        nc.tensor.matmul(
                            pQ, wkr[:, t, sl], i_sb[:, t, :],
                            start=False, stop=(t == NT - 1),
                        )
                    scr1 = scrp.tile([cw, 2 * D], mybir.dt.float32, tag=f"s{ci}")
                    nc.vector.scalar_tensor_tensor(
                        out=scr1[:], in0=pPQ[:], scalar=1.0, in1=kqc[ci][:],
                        op0=mult, op1=mult, accum_out=outc[ci][:, b : b + 1],
                    )
            for ci, (cs, cw) in enumerate(CHUNKS):
                nc.scalar.dma_start(out_d[cs : cs + cw, :], outc[ci][:])
    _split_multi_waits(nc)
    return nc


_NC = None


def _host_inputs(input_real, input_imag, kern):
    kr = np.ascontiguousarray(kern[:, :, 0]).astype(np.float32)  # [U, D]
    ki = np.ascontiguousarray(kern[:, :, 1]).astype(np.float32)
    wkr = np.ascontiguousarray(kr.T.reshape(NT, 128, U))
    wki = np.ascontiguousarray(ki.T.reshape(NT, 128, U))
    wkin = np.ascontiguousarray((-ki).T.reshape(NT, 128, U))
    kq = np.ascontiguousarray(np.concatenate([kr, -ki], axis=1))  # [U, 2D]
    maps = []
    for c in range(NCORES):
        sl = slice(c * BPC, (c + 1) * BPC)
        maps.append(
            {
                "zr": np.ascontiguousarray(input_real[sl]).astype(np.float32),
                "zi": np.ascontiguousarray(input_imag[sl]).astype(np.float32),
                "wkr": wkr,
                "wki": wki,
                "wkin": wkin,
                "kq": kq,
            }
        )
    return maps


def run(input_real, input_imag, kern, **run_kwargs):
    """Build (cached), run on 8 cores, return (output, BassKernelResults)."""
    global _NC
    from concourse.bass_utils import run_bass_kernel_spmd

    if _NC is None:
        _NC = build_nc()
    maps = _host_inputs(input_real, input_imag, kern)
    res = run_bass_kernel_spmd(_NC, maps, list(range(NCORES)), **run_kwargs)
    out = np.concatenate([res.results[c]["out"].T for c in range(NCORES)], axis=0)
    return np.ascontiguousarray(out, dtype=np.float32), res


def kernel(input_real, input_imag, kernel):
    out, _ = run(input_real, input_imag, kernel)
    return out


# revision 12
# speedup vs baseline: 1.0856x; 1.0856x over previous
# Trainium2 Bass kernel for nn_ComplexMeasurement: out[b,n] = Re(z_n^T Z_b z_n)
# with z = kr + i*ki (rows of the measurement kernel), Z = input_real + i*input_imag.
#
# Per batch b:  P = kr @ R_b - ki @ I_b   [U, D]   (PSUM-accumulated matmuls)
#               Q = ki @ R_b + kr @ I_b   [U, D]
#               out[b, n] = sum_j P[n,j]*kr[n,j] - Q[n,j]*ki[n,j]
#
# Sharding: data-parallel over batch, 16 batches per core on 8 cores.
# Stationary operands (kr^T, ki^T, -ki^T tiles) are pre-transposed on host.
import sys

for _p in ("/opt/trn_rl_repo", "/opt/trn_rl_repo/concourse"):
    if _p not in sys.path:
        sys.path.insert(0, _p)

import numpy as np

B, U, D = 128, 200, 512
NCORES = 8
BPC = B // NCORES  # batches per core
NT = D // 128  # contraction tiles
CHUNKS = ((0, 128), (128, 72))  # n-dim chunks (start, width)


def _split_multi_waits(nc, max_waits=1):
    # walrus in this env rejects instructions carrying >1 semaphore wait
    # ("Too many sync wait commands"). Move excess waits onto single-wait
    # NoOps inserted directly before the instruction on the same engine.
    import concourse.mybir as mybir

    n = 0
    for f in nc.m.functions:
        for bb in f.blocks:
            out = []
            changed = False
            for inst in bb.instructions:
                si = getattr(inst, "sync_info", None)
                waits = list(si.on_wait) if si is not None and si.on_wait else []
                if len(waits) > max_waits:
                    changed = True
                    extra, keep = waits[:-max_waits], waits[-max_waits:]
                    for w in extra:
                        n += 1
                        out.append(
                            mybir.InstNoOp(
                                name=f"WSPLIT-{n}",
                                engine=inst.engine,
                                ins=[],
                                outs=[],
                                sync_info=mybir.SyncInfo(on_wait=[w], on_update=[]),
                            )
                        )
                    inst.sync_info = mybir.SyncInfo(
                        on_wait=keep, on_update=list(si.on_update)
                    )
                out.append(inst)
            if changed:
                bb.instructions = out
    return n


def build_nc():
    import concourse.bass as bass
    import concourse.mybir as mybir
    import concourse.tile as tile

    f32 = mybir.dt.float32
    f32r = mybir.dt.float32r
    mult = mybir.AluOpType.mult
    add = mybir.AluOpType.add

    nc = bass.Bass()
    zr_d = nc.declare_dram_parameter("zr", [BPC, D, D], f32, isOutput=False)
    zi_d = nc.declare_dram_parameter("zi", [BPC, D, D], f32, isOutput=False)
    wkr_d = nc.declare_dram_parameter("wkr", [NT, 128, U], f32, isOutput=False)
    wki_d = nc.declare_dram_parameter("wki", [NT, 128, U], f32, isOutput=False)
    wkp_d = nc.declare_dram_parameter("wkp", [NT, 128, U], f32, isOutput=False)
    kq_d = nc.declare_dram_parameter("kq", [U, 3 * D], f32, isOutput=False)
    out_d = nc.declare_dram_parameter("out", [U, BPC], f32, isOutput=True)

    with tile.TileContext(nc) as tc:
        with (
            tc.tile_pool(name="const", bufs=1) as constp,
            tc.tile_pool(name="io", bufs=4) as iop,
            tc.tile_pool(name="scr", bufs=2) as scrp,
            tc.tile_pool(name="ps", bufs=2, space="PSUM") as psp,
        ):
            wkr = constp.tile([128, NT, U], f32r)
            wki = constp.tile([128, NT, U], f32r)
            wkp = constp.tile([128, NT, U], f32r)
            nc.sync.dma_start(wkr[:], wkr_d[:].bitcast(f32r).rearrange("t p n -> p t n"))
            nc.gpsimd.dma_start(wki[:], wki_d[:].bitcast(f32r).rearrange("t p n -> p t n"))
            nc.scalar.dma_start(wkp[:], wkp_d[:].bitcast(f32r).rearrange("t p n -> p t n"))
            kqc = {}
            outc = {}
            for ci, (cs, cw) in enumerate(CHUNKS):
                kqc[ci] = constp.tile([cw, 3 * D], f32, tag=f"kqc{ci}", name=f"kqc{ci}")
                nc.scalar.dma_start(kqc[ci][:], kq_d[cs : cs + cw, :])
                outc[ci] = constp.tile([cw, BPC], f32, tag=f"out{ci}", name=f"outc{ci}")

            addop = mybir.AluOpType.add
            for b in range(BPC):
                r_sb = iop.tile([128, NT, D], f32r, tag="r")
                i_sb = iop.tile([128, NT, D], f32r, tag="i")
                ri_sb = iop.tile([128, NT, D], f32r, tag="ri")
                r_eng = (nc.sync, nc.scalar, nc.sync, nc.gpsimd)
                i_eng = (nc.gpsimd, nc.sync, nc.gpsimd, nc.scalar)
                for t in range(NT):
                    r_eng[t].dma_start(
                        r_sb[:, t, :],
                        zr_d[b, 128 * t : 128 * (t + 1), :].bitcast(f32r),
                    )
                    i_eng[t].dma_start(
                        i_sb[:, t, :],
                        zi_d[b, 128 * t : 128 * (t + 1), :].bitcast(f32r),
                    )
                nc.vector.scalar_tensor_tensor(
                    out=ri_sb[:], in0=r_sb[:].bitcast(f32), scalar=1.0,
                    in1=i_sb[:].bitcast(f32), op0=mult, op1=addop,
                )
                for ci, (cs, cw) in enumerate(CHUNKS):
                    sl = slice(cs, cs + cw)
                    pM = psp.tile([cw, 3 * D], mybir.dt.float32, tag=f"PQ{ci}", bufs=1)
                    for t in range(NT):
                        nc.tensor.matmul(
                            pM[:, 0:D], wkr[:, t, sl],
                            r_sb[:, t, :],
                            start=(t == 0), stop=(t == NT - 1),
                        )
                    for t in range(NT):
                        nc.tensor.matmul(
                            pM[:, D : 2 * D], wki[:, t, sl],
                            i_sb[:, t, :],
                            start=(t == 0), stop=(t == NT - 1),
                        )
                    for t in range(NT):
                        nc.tensor.matmul(
                            pM[:, 2 * D : 3 * D], wkp[:, t, sl],
                            ri_sb[:, t, :],
                            start=(t == 0), stop=(t == NT - 1),
                        )
                    scr1 = scrp.tile([cw, 3 * D], mybir.dt.float32, tag=f"s{ci}")
                    nc.vector.scalar_tensor_tensor(
                        out=scr1[:], in0=pM[:], scalar=1.0, in1=kqc[ci][:],
                        op0=mult, op1=mult, accum_out=outc[ci][:, b : b + 1],
                    )
            for ci, (cs, cw) in enumerate(CHUNKS):
                nc.scalar.dma_start(out_d[cs : cs + cw, :], outc[ci][:])
    _split_multi_waits(nc)
    return nc


_NC = None


def _host_inputs(input_real, input_imag, kern):
    kr = np.ascontiguousarray(kern[:, :, 0]).astype(np.float32)  # [U, D]
    ki = np.ascontiguousarray(kern[:, :, 1]).astype(np.float32)
    wkr = np.ascontiguousarray(kr.T.reshape(NT, 128, U))
    wki = np.ascontiguousarray(ki.T.reshape(NT, 128, U))
    wkp = np.ascontiguousarray((kr + ki).T.reshape(NT, 128, U))
    # out[b,n] = sum_j m1*(kr+ki) + m2*(ki-kr) - m3*ki   (m1=kr@R, m2=ki@I, m3=(kr+ki)@(R+I))
    kq = np.ascontiguousarray(np.concatenate([kr + ki, ki - kr, -ki], axis=1))  # [U, 3D]
    maps = []
    for c in range(NCORES):
        sl = slice(c * BPC, (c + 1) * BPC)
        maps.append(
            {
                "zr": np.ascontiguousarray(input_real[sl]).astype(np.float32),
                "zi": np.ascontiguousarray(input_imag[sl]).astype(np.float32),
                "wkr": wkr,
                "wki": wki,
                "wkp": wkp,
                "kq": kq,
            }
        )
    return maps


def run(input_real, input_imag, kern, **run_kwargs):
    """Build (cached), run on 8 cores, return (output, BassKernelResults)."""
    global _NC
    from concourse.bass_utils import run_bass_kernel_spmd

    if _NC is None:
        _NC = build_nc()
    maps = _host_inputs(input_real, input_imag, kern)
    res = run_bass_kernel_spmd(_NC, maps, list(range(NCORES)), **run_kwargs)
    out = np.concatenate([res.results[c]["out"].T for c in range(NCORES)], axis=0)
    return np.ascontiguousarray(out, dtype=np.float32), res


def kernel(input_real, input_imag, kernel):
    out, _ = run(input_real, input_imag, kernel)
    return out
